# revision 1
# baseline (speedup 1.0000x reference)
"""TPS GridGenerator kernel for Trainium2 (8 NeuronCores).

Math: the reference builds a fixed (30,30) TPS system matrix L and a fixed
(7680,30) phi matrix (both batch-independent), solves L @ coeffs = [scp; 0]
per batch, and returns phi @ coeffs reshaped to (B, 48, 160, 2).

The tiny 30x30 solve is replicated bit-exactly on host via the same
jnp.linalg.solve on the CPU backend (L is exactly singular - all control
points lie on y^2-1=0, which the cubic polynomial tail can represent - so
the solve's result, NaN or junk, is implementation-defined; running the
identical op reproduces it). The memory-bound einsum phi @ coeffs (99.9% of
the traffic) runs on the 8 cores, grid-sharded 7680 -> 8 x 960.

Device formulation per core k (grid rows [960k, 960k+960)):
    out[b, 2g+c] = sum_j ctT[j, b] * W[j, 2g+c]
with j = 2n+c' (60 terms), ctT = coeffs.reshape(128,60).T replicated, and
W[2n+c', 2g+c] = phi[960k+g, n] * (c==c') per-core constant. One stationary
(60x128) matmul per 480-column chunk writes PSUM already in the final
(batch, grid, xy)-interleaved layout, so the output DMA is 128 descriptors
of 7.5KB contiguous each.
"""

import numpy as np

B = 128
NUM_FID = 30  # 20 kernel + 10 polynomial terms
IMG_H = 48
IMG_W = 160
HW = IMG_H * IMG_W  # 7680
N_CORES = 8
G_SH = HW // N_CORES  # 960
JC = 2 * NUM_FID  # 60 contraction terms (n, c) interleaved
NOUT = 2 * G_SH  # 1920 output cols per core
CHUNK = 480
N_CHUNKS = NOUT // CHUNK  # 4

_cache = {}


def _jax_cpu():
    import jax

    try:
        return jax.devices("cpu")[0]
    except Exception:
        return None


def _host_constants():
    """L (30,30) and phi (7680,30), computed exactly as the reference does
    (same jnp ops, f32, CPU backend) so downstream bits match."""
    if "L" in _cache:
        return _cache["L"], _cache["phi"]
    import jax
    import jax.numpy as jnp
    from contextlib import nullcontext

    cpu = _jax_cpu()
    with jax.default_device(cpu) if cpu is not None else nullcontext():
        npe = 10
        tx = jnp.linspace(-1.0, 1.0, npe)
        top = jnp.stack([tx, -jnp.ones(npe)], axis=1)
        bot = jnp.stack([tx, jnp.ones(npe)], axis=1)
        C = jnp.concatenate([top, bot], axis=0)  # (20, 2)

        def P_matrix(pts):
            x = pts[..., 0:1]
            y = pts[..., 1:2]
            one = jnp.ones_like(x)
            return jnp.concatenate(
                [one, x, y, x * x, x * y, y * y, x * x * x, x * x * y, x * y * y, y * y * y],
                axis=-1,
            )

        def K_matrix(p1, p2):
            diff = p1[:, None, :] - p2[None, :, :]
            r2 = jnp.clip(jnp.sum(diff * diff, axis=-1), 1e-8, None)
            return r2 * (0.5 * jnp.log(r2))

        K = K_matrix(C, C)
        P = P_matrix(C)
        L_top = jnp.concatenate([K, P], axis=1)
        L_bot = jnp.concatenate([P.T, jnp.zeros((10, 10), P.dtype)], axis=1)
        L = jnp.concatenate([L_top, L_bot], axis=0)  # (30, 30)

        gy = jnp.linspace(-1.0, 1.0, IMG_H)
        gx = jnp.linspace(-1.0, 1.0, IMG_W)
        xx, yy = jnp.meshgrid(gx, gy)
        G = jnp.stack([xx, yy], axis=-1).reshape(HW, 2)
        phi = jnp.concatenate([K_matrix(G, C), P_matrix(G)], axis=1)  # (7680, 30)

        L = np.asarray(L, dtype=np.float32)
        phi = np.asarray(phi, dtype=np.float32)
    _cache["L"] = L
    _cache["phi"] = phi
    return L, phi


def _w_shards():
    """Per-core W (60, 1920): W[2n+c, 2g+c] = phi[960k+g, n]."""
    if "w" in _cache:
        return _cache["w"]
    _, phi = _host_constants()
    ws = []
    for k in range(N_CORES):
        phik = phi[k * G_SH : (k + 1) * G_SH]  # (960, 30)
        w4 = np.zeros((NUM_FID, 2, G_SH, 2), dtype=np.float32)
        w4[:, 0, :, 0] = phik.T
        w4[:, 1, :, 1] = phik.T
        ws.append(np.ascontiguousarray(w4.reshape(JC, NOUT)))
    _cache["w"] = ws
    return ws


def _coeffs(scp):
    """Replicate the reference's batched solve on jax-CPU bit-for-bit."""
    import jax
    import jax.numpy as jnp
    from contextlib import nullcontext

    L, _ = _host_constants()
    cpu = _jax_cpu()
    with jax.default_device(cpu) if cpu is not None else nullcontext():
        target = jnp.concatenate(
            [jnp.asarray(scp), jnp.zeros((scp.shape[0], 10, 2), jnp.float32)], axis=1
        )
        co = jnp.linalg.solve(jnp.asarray(L)[None], target)  # (B, 30, 2)
        return np.asarray(co, dtype=np.float32)


def _build_nc():
    if "nc" in _cache:
        return _cache["nc"]
    import concourse.bacc as bacc
    import concourse.mybir as mybir
    import concourse.tile as tile

    nc = bacc.Bacc("TRN2", target_bir_lowering=False, debug=False)
    f32 = mybir.dt.float32
    ct_dram = nc.dram_tensor("ct", [JC, B], f32, kind="ExternalInput")
    w_dram = nc.dram_tensor("w", [JC, NOUT], f32, kind="ExternalInput")
    out_dram = nc.dram_tensor("out", [B, NOUT], f32, kind="ExternalOutput")

    with tile.TileContext(nc) as tc:
        with (
            tc.tile_pool(name="sbuf", bufs=1) as pool,
            tc.tile_pool(name="psum", bufs=1, space="PSUM") as psum,
        ):
            ct = pool.tile([JC, B], f32, tag="ct")
            nc.sync.dma_start(out=ct[:], in_=ct_dram[:, :])
            for i in range(N_CHUNKS):
                sl = slice(i * CHUNK, (i + 1) * CHUNK)
                wt = pool.tile([JC, CHUNK], f32, tag=f"w{i}")
                nc.sync.dma_start(out=wt[:], in_=w_dram[:, sl])
                ps = psum.tile([B, CHUNK], f32, tag=f"ps{i}")
                nc.tensor.matmul(ps[:], ct[:], wt[:], start=True, stop=True)
                ot = pool.tile([B, CHUNK], f32, tag=f"o{i}")
                nc.any.tensor_copy(ot[:], ps[:])
                nc.sync.dma_start(out=out_dram[:, sl], in_=ot[:])
    nc.compile()
    _cache["nc"] = nc
    return nc


def kernel(source_control_points: np.ndarray) -> np.ndarray:
    scp = np.ascontiguousarray(source_control_points, dtype=np.float32)
    co = _coeffs(scp)  # (B, 30, 2)
    ctT = np.ascontiguousarray(co.reshape(scp.shape[0], JC).T)  # (60, B)
    ws = _w_shards()
    nc = _build_nc()

    from concourse.bass_utils import run_bass_kernel_spmd

    in_maps = [{"ct": ctT, "w": ws[k]} for k in range(N_CORES)]
    res = run_bass_kernel_spmd(nc, in_maps, core_ids=list(range(N_CORES)))
    parts = [res.results[k]["out"].reshape(scp.shape[0], G_SH, 2) for k in range(N_CORES)]
    full = np.concatenate(parts, axis=1)  # (B, 7680, 2)
    return full.reshape(scp.shape[0], IMG_H, IMG_W, 2)


# revision 8
# speedup vs baseline: 1.2852x; 1.2852x over previous
"""TPS GridGenerator kernel for Trainium2 (8 NeuronCores).

Math: the reference builds a fixed (30,30) TPS system matrix L and a fixed
(7680,30) phi matrix (both batch-independent), solves L @ coeffs = [scp; 0]
per batch, and returns phi @ coeffs reshaped to (B, 48, 160, 2).

The tiny 30x30 solve is replicated bit-exactly on host via the same
jnp.linalg.solve on the CPU backend (L is exactly singular - all control
points lie on y^2-1=0, which the cubic polynomial tail can represent - so
the solve's result is implementation-defined; running the identical op
reproduces it). The memory-bound einsum phi @ coeffs (99.9% of the traffic)
runs on the 8 cores, grid-sharded 7680 -> 8 x 960.

Device program per core k (grid rows [960k, 960k+960)):
  - ct  (60,128) SBUF: rows 0-29 = coeffs[:,:,0].T, rows 30-59 = coeffs[:,:,1].T
  - phit (30,960) SBUF: phi[960k:960k+960].T, in 2 chunks of 480 cols
  - 4 float32r matmuls: psum_x/y[i] (128,480) = ct[x|y].T @ phit_chunk[i]
  - stride-2 PSUM->SBUF copies interleave x,y into out (128,1920) so the
    output DMA lands in final (batch, grid, xy) DRAM layout with large
    contiguous descriptors.
"""

import numpy as np

B = 128
NUM_FID = 30  # 20 kernel + 10 polynomial terms
IMG_H = 48
IMG_W = 160
HW = IMG_H * IMG_W  # 7680
N_CORES = 8
G_SH = HW // N_CORES  # 960
NOUT = 2 * G_SH  # 1920 output cols per core
CHUNK = 480
N_CHUNKS = G_SH // CHUNK  # 2

_cache = {}


def _jax_cpu():
    import jax

    try:
        return jax.devices("cpu")[0]
    except Exception:
        return None


def _host_constants():
    """L (30,30) and phi (7680,30), computed exactly as the reference does
    (same jnp ops, f32, CPU backend) so downstream bits match."""
    if "L" in _cache:
        return _cache["L"], _cache["phi"]
    import jax
    import jax.numpy as jnp
    from contextlib import nullcontext

    cpu = _jax_cpu()
    with jax.default_device(cpu) if cpu is not None else nullcontext():
        npe = 10
        tx = jnp.linspace(-1.0, 1.0, npe)
        top = jnp.stack([tx, -jnp.ones(npe)], axis=1)
        bot = jnp.stack([tx, jnp.ones(npe)], axis=1)
        C = jnp.concatenate([top, bot], axis=0)  # (20, 2)

        def P_matrix(pts):
            x = pts[..., 0:1]
            y = pts[..., 1:2]
            one = jnp.ones_like(x)
            return jnp.concatenate(
                [one, x, y, x * x, x * y, y * y, x * x * x, x * x * y, x * y * y, y * y * y],
                axis=-1,
            )

        def K_matrix(p1, p2):
            diff = p1[:, None, :] - p2[None, :, :]
            r2 = jnp.clip(jnp.sum(diff * diff, axis=-1), 1e-8, None)
            return r2 * (0.5 * jnp.log(r2))

        K = K_matrix(C, C)
        P = P_matrix(C)
        L_top = jnp.concatenate([K, P], axis=1)
        L_bot = jnp.concatenate([P.T, jnp.zeros((10, 10), P.dtype)], axis=1)
        L = jnp.concatenate([L_top, L_bot], axis=0)  # (30, 30)

        gy = jnp.linspace(-1.0, 1.0, IMG_H)
        gx = jnp.linspace(-1.0, 1.0, IMG_W)
        xx, yy = jnp.meshgrid(gx, gy)
        G = jnp.stack([xx, yy], axis=-1).reshape(HW, 2)
        phi = jnp.concatenate([K_matrix(G, C), P_matrix(G)], axis=1)  # (7680, 30)

        L = np.asarray(L, dtype=np.float32)
        phi = np.asarray(phi, dtype=np.float32)
    _cache["L"] = L
    _cache["phi"] = phi
    return L, phi


def _phit_shards():
    """Per-core phi[960k:960k+960].T as contiguous (30, 960) f32."""
    if "phit" in _cache:
        return _cache["phit"]
    _, phi = _host_constants()
    _cache["phit"] = [
        np.ascontiguousarray(phi[k * G_SH : (k + 1) * G_SH].T) for k in range(N_CORES)
    ]
    return _cache["phit"]


def _coeffs(scp):
    """Replicate the reference's batched solve on jax-CPU bit-for-bit."""
    import jax
    import jax.numpy as jnp
    from contextlib import nullcontext

    L, _ = _host_constants()
    cpu = _jax_cpu()
    with jax.default_device(cpu) if cpu is not None else nullcontext():
        target = jnp.concatenate(
            [jnp.asarray(scp), jnp.zeros((scp.shape[0], 10, 2), jnp.float32)], axis=1
        )
        co = jnp.linalg.solve(jnp.asarray(L)[None], target)  # (B, 30, 2)
        return np.asarray(co, dtype=np.float32)


def _build_nc():
    if "nc" in _cache:
        return _cache["nc"]
    import concourse.bacc as bacc
    import concourse.mybir as mybir
    import concourse.tile as tile

    nc = bacc.Bacc("TRN2", target_bir_lowering=False, debug=False)
    f32 = mybir.dt.float32
    f32r = mybir.dt.float32r
    ct_dram = nc.dram_tensor("ct", [2, NUM_FID, B], f32, kind="ExternalInput")
    pt_dram = nc.dram_tensor("phit", [NUM_FID, G_SH], f32, kind="ExternalInput")
    out_dram = nc.dram_tensor("out", [B, NOUT], f32, kind="ExternalOutput")

    with tile.TileContext(nc) as tc:
        with (
            tc.tile_pool(name="sbuf", bufs=1) as pool,
            tc.tile_pool(name="psum", bufs=1, space="PSUM") as psum,
        ):
            # float32r operands run the PE at 1 cycle/row (vs 4 for fp32);
            # the verifier requires the producer to emit fp32r, so the loads
            # are gpsimd casting DMAs (f32 -> f32r truncation).
            ctx_t = pool.tile([NUM_FID, B], f32r, tag="ctx")
            cty_t = pool.tile([NUM_FID, B], f32r, tag="cty")
            nc.gpsimd.dma_start(out=ctx_t[:], in_=ct_dram[0])
            nc.gpsimd.dma_start(out=cty_t[:], in_=ct_dram[1])
            pt = pool.tile([NUM_FID, G_SH], f32r, tag="pt")
            nc.gpsimd.dma_start(out=pt[:], in_=pt_dram[:, :])
            for i in range(N_CHUNKS):
                csl = slice(i * CHUNK, (i + 1) * CHUNK)
                psx = psum.tile([B, CHUNK], f32, tag=f"psx{i}")
                psy = psum.tile([B, CHUNK], f32, tag=f"psy{i}")
                nc.tensor.matmul(psx[:], ctx_t[:], pt[:, csl], start=True, stop=True)
                nc.tensor.matmul(psy[:], cty_t[:], pt[:, csl], start=True, stop=True)
                ot = pool.tile([B, 2 * CHUNK], f32, tag=f"o{i}")
                nc.vector.tensor_copy(ot[:, 0 : 2 * CHUNK : 2], psx[:])
                nc.scalar.copy(ot[:, 1 : 2 * CHUNK : 2], psy[:])
                nc.sync.dma_start(
                    out=out_dram[:, 2 * CHUNK * i : 2 * CHUNK * (i + 1)], in_=ot[:]
                )
    nc.compile()
    _cache["nc"] = nc
    return nc


def kernel(source_control_points: np.ndarray) -> np.ndarray:
    scp = np.ascontiguousarray(source_control_points, dtype=np.float32)
    co = _coeffs(scp)  # (B, 30, 2)
    nb = scp.shape[0]
    # ct[c, n, b] = coeffs[b, n, c]
    ct = np.ascontiguousarray(co.transpose(2, 1, 0))
    pts = _phit_shards()
    nc = _build_nc()

    from concourse.bass_utils import run_bass_kernel_spmd

    in_maps = [{"ct": ct, "phit": pts[k]} for k in range(N_CORES)]
    res = run_bass_kernel_spmd(nc, in_maps, core_ids=list(range(N_CORES)))
    parts = [res.results[k]["out"].reshape(nb, G_SH, 2) for k in range(N_CORES)]
    full = np.concatenate(parts, axis=1)  # (B, 7680, 2)
    return full.reshape(nb, IMG_H, IMG_W, 2)


# revision 10
# speedup vs baseline: 1.6680x; 1.2979x over previous
"""TPS GridGenerator kernel for Trainium2 (8 NeuronCores).

Math: the reference builds a fixed (30,30) TPS system matrix L and a fixed
(7680,30) phi matrix (both batch-independent), solves L @ coeffs = [scp; 0]
per batch, and returns phi @ coeffs reshaped to (B, 48, 160, 2).

The tiny 30x30 solve is replicated bit-exactly on host via the same
jnp.linalg.solve on the CPU backend (L is exactly singular - all control
points lie on y^2-1=0, which the cubic polynomial tail can represent - so
the solve's result is implementation-defined; running the identical op
reproduces it). The memory-bound einsum phi @ coeffs (99.9% of the traffic)
runs on the 8 cores, grid-sharded 7680 -> 8 x 960.

Device program per core k (grid rows [960k, 960k+960)):
  - ct  (60,128) SBUF: rows 0-29 = coeffs[:,:,0].T, rows 30-59 = coeffs[:,:,1].T
  - phit (30,960) SBUF: phi[960k:960k+960].T, in 2 chunks of 480 cols
  - 4 float32r matmuls: psum_x/y[i] (128,480) = ct[x|y].T @ phit_chunk[i]
  - stride-2 PSUM->SBUF copies interleave x,y into out (128,1920) so the
    output DMA lands in final (batch, grid, xy) DRAM layout with large
    contiguous descriptors.
"""

import numpy as np

B = 128
NUM_FID = 30  # 20 kernel + 10 polynomial terms
IMG_H = 48
IMG_W = 160
HW = IMG_H * IMG_W  # 7680
N_CORES = 8
G_SH = HW // N_CORES  # 960
NOUT = 2 * G_SH  # 1920 output cols per core
CHUNK = 480
N_CHUNKS = G_SH // CHUNK  # 2

_cache = {}


def _jax_cpu():
    import jax

    try:
        return jax.devices("cpu")[0]
    except Exception:
        return None


def _host_constants():
    """L (30,30) and phi (7680,30), computed exactly as the reference does
    (same jnp ops, f32, CPU backend) so downstream bits match."""
    if "L" in _cache:
        return _cache["L"], _cache["phi"]
    import jax
    import jax.numpy as jnp
    from contextlib import nullcontext

    cpu = _jax_cpu()
    with jax.default_device(cpu) if cpu is not None else nullcontext():
        npe = 10
        tx = jnp.linspace(-1.0, 1.0, npe)
        top = jnp.stack([tx, -jnp.ones(npe)], axis=1)
        bot = jnp.stack([tx, jnp.ones(npe)], axis=1)
        C = jnp.concatenate([top, bot], axis=0)  # (20, 2)

        def P_matrix(pts):
            x = pts[..., 0:1]
            y = pts[..., 1:2]
            one = jnp.ones_like(x)
            return jnp.concatenate(
                [one, x, y, x * x, x * y, y * y, x * x * x, x * x * y, x * y * y, y * y * y],
                axis=-1,
            )

        def K_matrix(p1, p2):
            diff = p1[:, None, :] - p2[None, :, :]
            r2 = jnp.clip(jnp.sum(diff * diff, axis=-1), 1e-8, None)
            return r2 * (0.5 * jnp.log(r2))

        K = K_matrix(C, C)
        P = P_matrix(C)
        L_top = jnp.concatenate([K, P], axis=1)
        L_bot = jnp.concatenate([P.T, jnp.zeros((10, 10), P.dtype)], axis=1)
        L = jnp.concatenate([L_top, L_bot], axis=0)  # (30, 30)

        gy = jnp.linspace(-1.0, 1.0, IMG_H)
        gx = jnp.linspace(-1.0, 1.0, IMG_W)
        xx, yy = jnp.meshgrid(gx, gy)
        G = jnp.stack([xx, yy], axis=-1).reshape(HW, 2)
        phi = jnp.concatenate([K_matrix(G, C), P_matrix(G)], axis=1)  # (7680, 30)

        L = np.asarray(L, dtype=np.float32)
        phi = np.asarray(phi, dtype=np.float32)
    _cache["L"] = L
    _cache["phi"] = phi
    return L, phi


def _phit_shards():
    """Per-core phi[960k:960k+960].T as contiguous (30, 960) f32."""
    if "phit" in _cache:
        return _cache["phit"]
    _, phi = _host_constants()
    _cache["phit"] = [
        np.ascontiguousarray(phi[k * G_SH : (k + 1) * G_SH].T) for k in range(N_CORES)
    ]
    return _cache["phit"]


def _coeffs(scp):
    """Replicate the reference's batched solve on jax-CPU bit-for-bit."""
    import jax
    import jax.numpy as jnp
    from contextlib import nullcontext

    L, _ = _host_constants()
    cpu = _jax_cpu()
    with jax.default_device(cpu) if cpu is not None else nullcontext():
        target = jnp.concatenate(
            [jnp.asarray(scp), jnp.zeros((scp.shape[0], 10, 2), jnp.float32)], axis=1
        )
        co = jnp.linalg.solve(jnp.asarray(L)[None], target)  # (B, 30, 2)
        return np.asarray(co, dtype=np.float32)


def _build_nc():
    if "nc" in _cache:
        return _cache["nc"]
    import concourse.bacc as bacc
    import concourse.mybir as mybir
    import concourse.tile as tile

    nc = bacc.Bacc("TRN2", target_bir_lowering=False, debug=False)
    f32 = mybir.dt.float32
    f32r = mybir.dt.float32r
    # One fused input: cols [0:128]=coeffs_x.T, [128:256]=coeffs_y.T,
    # [256:1216]=phi_shard.T. Declared float32r (same bits as f32) so a
    # single non-casting HWDGE DMA satisfies the fp32r-producer rule and
    # the PE runs at 1 cycle/row instead of fp32's 4.
    inp_dram = nc.dram_tensor("inp", [NUM_FID, 2 * B + G_SH], f32r, kind="ExternalInput")
    out_dram = nc.dram_tensor("out", [B, NOUT], f32, kind="ExternalOutput")

    with tile.TileContext(nc) as tc:
        with (
            tc.tile_pool(name="sbuf", bufs=1) as pool,
            tc.tile_pool(name="psum", bufs=1, space="PSUM") as psum,
        ):
            inp = pool.tile([NUM_FID, 2 * B + G_SH], f32r, tag="inp")
            nc.sync.dma_start(out=inp[:], in_=inp_dram[:, :])
            ctx_t = inp[:, 0:B]
            cty_t = inp[:, B : 2 * B]
            for i in range(N_CHUNKS):
                csl = slice(2 * B + i * CHUNK, 2 * B + (i + 1) * CHUNK)
                psx = psum.tile([B, CHUNK], f32, tag=f"psx{i}")
                psy = psum.tile([B, CHUNK], f32, tag=f"psy{i}")
                nc.tensor.matmul(psx[:], ctx_t, inp[:, csl], start=True, stop=True)
                nc.tensor.matmul(psy[:], cty_t, inp[:, csl], start=True, stop=True)
                ot = pool.tile([B, 2 * CHUNK], f32, tag=f"o{i}")
                nc.vector.tensor_copy(ot[:, 0 : 2 * CHUNK : 2], psx[:])
                nc.scalar.copy(ot[:, 1 : 2 * CHUNK : 2], psy[:])
                nc.sync.dma_start(
                    out=out_dram[:, 2 * CHUNK * i : 2 * CHUNK * (i + 1)], in_=ot[:]
                )
    nc.compile()
    _cache["nc"] = nc
    return nc


def kernel(source_control_points: np.ndarray) -> np.ndarray:
    scp = np.ascontiguousarray(source_control_points, dtype=np.float32)
    co = _coeffs(scp)  # (B, 30, 2)
    nb = scp.shape[0]
    pts = _phit_shards()
    nc = _build_nc()
    # fused input per core: [coeffs_x.T | coeffs_y.T | phi_shard.T]
    inps = []
    for k in range(N_CORES):
        m = np.empty((NUM_FID, 2 * B + G_SH), dtype=np.float32)
        m[:, 0:B] = co[:, :, 0].T
        m[:, B : 2 * B] = co[:, :, 1].T
        m[:, 2 * B :] = pts[k]
        inps.append(m)

    from concourse.bass_utils import run_bass_kernel_spmd

    in_maps = [{"inp": inps[k]} for k in range(N_CORES)]
    res = run_bass_kernel_spmd(nc, in_maps, core_ids=list(range(N_CORES)))
    parts = [res.results[k]["out"].reshape(nb, G_SH, 2) for k in range(N_CORES)]
    full = np.concatenate(parts, axis=1)  # (B, 7680, 2)
    return full.reshape(nb, IMG_H, IMG_W, 2)


# revision 12
# speedup vs baseline: 1.7346x; 1.0400x over previous
"""TPS GridGenerator kernel for Trainium2 (8 NeuronCores).

Math: the reference builds a fixed (30,30) TPS system matrix L and a fixed
(7680,30) phi matrix (both batch-independent), solves L @ coeffs = [scp; 0]
per batch, and returns phi @ coeffs reshaped to (B, 48, 160, 2).

The tiny 30x30 solve is replicated bit-exactly on host via the same
jnp.linalg.solve on the CPU backend (L is exactly singular - all control
points lie on y^2-1=0, which the cubic polynomial tail can represent - so
the solve's result is implementation-defined; running the identical op
reproduces it). The memory-bound einsum phi @ coeffs (99.9% of the traffic)
runs on the 8 cores, grid-sharded 7680 -> 8 x 960.

Device program per core k (grid rows [960k, 960k+960)):
  - ct  (60,128) SBUF: rows 0-29 = coeffs[:,:,0].T, rows 30-59 = coeffs[:,:,1].T
  - phit (30,960) SBUF: phi[960k:960k+960].T, in 2 chunks of 480 cols
  - 4 float32r matmuls: psum_x/y[i] (128,480) = ct[x|y].T @ phit_chunk[i]
  - stride-2 PSUM->SBUF copies interleave x,y into out (128,1920) so the
    output DMA lands in final (batch, grid, xy) DRAM layout with large
    contiguous descriptors.
"""

import numpy as np

B = 128
NUM_FID = 30  # 20 kernel + 10 polynomial terms
IMG_H = 48
IMG_W = 160
HW = IMG_H * IMG_W  # 7680
N_CORES = 8
G_SH = HW // N_CORES  # 960
NOUT = 2 * G_SH  # 1920 output cols per core
CHUNK = 320  # >=256 keeps float32r at 1 cycle/row; 3 chunks pipeline stores
N_CHUNKS = G_SH // CHUNK

_cache = {}


def _jax_cpu():
    import jax

    try:
        return jax.devices("cpu")[0]
    except Exception:
        return None


def _host_constants():
    """L (30,30) and phi (7680,30), computed exactly as the reference does
    (same jnp ops, f32, CPU backend) so downstream bits match."""
    if "L" in _cache:
        return _cache["L"], _cache["phi"]
    import jax
    import jax.numpy as jnp
    from contextlib import nullcontext

    cpu = _jax_cpu()
    with jax.default_device(cpu) if cpu is not None else nullcontext():
        npe = 10
        tx = jnp.linspace(-1.0, 1.0, npe)
        top = jnp.stack([tx, -jnp.ones(npe)], axis=1)
        bot = jnp.stack([tx, jnp.ones(npe)], axis=1)
        C = jnp.concatenate([top, bot], axis=0)  # (20, 2)

        def P_matrix(pts):
            x = pts[..., 0:1]
            y = pts[..., 1:2]
            one = jnp.ones_like(x)
            return jnp.concatenate(
                [one, x, y, x * x, x * y, y * y, x * x * x, x * x * y, x * y * y, y * y * y],
                axis=-1,
            )

        def K_matrix(p1, p2):
            diff = p1[:, None, :] - p2[None, :, :]
            r2 = jnp.clip(jnp.sum(diff * diff, axis=-1), 1e-8, None)
            return r2 * (0.5 * jnp.log(r2))

        K = K_matrix(C, C)
        P = P_matrix(C)
        L_top = jnp.concatenate([K, P], axis=1)
        L_bot = jnp.concatenate([P.T, jnp.zeros((10, 10), P.dtype)], axis=1)
        L = jnp.concatenate([L_top, L_bot], axis=0)  # (30, 30)

        gy = jnp.linspace(-1.0, 1.0, IMG_H)
        gx = jnp.linspace(-1.0, 1.0, IMG_W)
        xx, yy = jnp.meshgrid(gx, gy)
        G = jnp.stack([xx, yy], axis=-1).reshape(HW, 2)
        phi = jnp.concatenate([K_matrix(G, C), P_matrix(G)], axis=1)  # (7680, 30)

        L = np.asarray(L, dtype=np.float32)
        phi = np.asarray(phi, dtype=np.float32)
    _cache["L"] = L
    _cache["phi"] = phi
    return L, phi


def _phit_shards():
    """Per-core phi[960k:960k+960].T as contiguous (30, 960) f32."""
    if "phit" in _cache:
        return _cache["phit"]
    _, phi = _host_constants()
    _cache["phit"] = [
        np.ascontiguousarray(phi[k * G_SH : (k + 1) * G_SH].T) for k in range(N_CORES)
    ]
    return _cache["phit"]


def _coeffs(scp):
    """Replicate the reference's batched solve on jax-CPU bit-for-bit."""
    import jax
    import jax.numpy as jnp
    from contextlib import nullcontext

    L, _ = _host_constants()
    cpu = _jax_cpu()
    with jax.default_device(cpu) if cpu is not None else nullcontext():
        target = jnp.concatenate(
            [jnp.asarray(scp), jnp.zeros((scp.shape[0], 10, 2), jnp.float32)], axis=1
        )
        co = jnp.linalg.solve(jnp.asarray(L)[None], target)  # (B, 30, 2)
        return np.asarray(co, dtype=np.float32)


def _build_nc():
    if "nc" in _cache:
        return _cache["nc"]
    import concourse.bacc as bacc
    import concourse.mybir as mybir
    import concourse.tile as tile

    nc = bacc.Bacc("TRN2", target_bir_lowering=False, debug=False)
    f32 = mybir.dt.float32
    f32r = mybir.dt.float32r
    # One fused input: cols [0:128]=coeffs_x.T, [128:256]=coeffs_y.T,
    # [256:1216]=phi_shard.T. Declared float32r (same bits as f32) so a
    # single non-casting HWDGE DMA satisfies the fp32r-producer rule and
    # the PE runs at 1 cycle/row instead of fp32's 4.
    inp_dram = nc.dram_tensor("inp", [NUM_FID, 2 * B + G_SH], f32r, kind="ExternalInput")
    out_dram = nc.dram_tensor("out", [B, NOUT], f32, kind="ExternalOutput")

    with tile.TileContext(nc) as tc:
        with (
            tc.tile_pool(name="sbuf", bufs=1) as pool,
            tc.tile_pool(name="psum", bufs=1, space="PSUM") as psum,
        ):
            inp = pool.tile([NUM_FID, 2 * B + G_SH], f32r, tag="inp")
            nc.sync.dma_start(out=inp[:], in_=inp_dram[:, :])
            ctx_t = inp[:, 0:B]
            cty_t = inp[:, B : 2 * B]
            for i in range(N_CHUNKS):
                csl = slice(2 * B + i * CHUNK, 2 * B + (i + 1) * CHUNK)
                psx = psum.tile([B, CHUNK], f32, tag=f"psx{i}")
                psy = psum.tile([B, CHUNK], f32, tag=f"psy{i}")
                nc.tensor.matmul(psx[:], ctx_t, inp[:, csl], start=True, stop=True)
                nc.tensor.matmul(psy[:], cty_t, inp[:, csl], start=True, stop=True)
                ot = pool.tile([B, 2 * CHUNK], f32, tag=f"o{i}")
                nc.vector.tensor_copy(ot[:, 0 : 2 * CHUNK : 2], psx[:])
                nc.scalar.copy(ot[:, 1 : 2 * CHUNK : 2], psy[:])
                # alternate issuing engine so store issue latency overlaps
                st_eng = nc.sync if i % 2 == 0 else nc.scalar
                st_eng.dma_start(
                    out=out_dram[:, 2 * CHUNK * i : 2 * CHUNK * (i + 1)], in_=ot[:]
                )
    nc.compile()
    _cache["nc"] = nc
    return nc


def kernel(source_control_points: np.ndarray) -> np.ndarray:
    scp = np.ascontiguousarray(source_control_points, dtype=np.float32)
    co = _coeffs(scp)  # (B, 30, 2)
    nb = scp.shape[0]
    pts = _phit_shards()
    nc = _build_nc()
    # fused input per core: [coeffs_x.T | coeffs_y.T | phi_shard.T]
    inps = []
    for k in range(N_CORES):
        m = np.empty((NUM_FID, 2 * B + G_SH), dtype=np.float32)
        m[:, 0:B] = co[:, :, 0].T
        m[:, B : 2 * B] = co[:, :, 1].T
        m[:, 2 * B :] = pts[k]
        inps.append(m)

    from concourse.bass_utils import run_bass_kernel_spmd

    in_maps = [{"inp": inps[k]} for k in range(N_CORES)]
    res = run_bass_kernel_spmd(nc, in_maps, core_ids=list(range(N_CORES)))
    parts = [res.results[k]["out"].reshape(nb, G_SH, 2) for k in range(N_CORES)]
    full = np.concatenate(parts, axis=1)  # (B, 7680, 2)
    return full.reshape(nb, IMG_H, IMG_W, 2)
